# revision 1
# baseline (speedup 1.0000x reference)
"""3-layer GCN (GCNConv x3) on Trainium2, distributed across 8 NeuronCores.

Strategy (graph/data parallel, per the sharding hint):
  - Nodes are block-partitioned across the 8 cores; each core owns the
    destination side (scatter-add aggregation) for its node shard.
  - The tiny weight matrices are replicated; per layer each core computes
    g = dis * h for its shard, the shards are AllGathered into a shared HBM
    table G, and each core aggregates messages for its own nodes with
    dma_gather (256B rows from G) + dma_scatter_add (CCE-add, SBUF
    destination in the parity-split layout).
  - Symmetric normalization is folded per-node:
       (Ahat h)[c] = dis[c] * ( sum_{e->c} dis[r] h[r] + dis[c] h[c] )
    so no per-edge multiplies are needed on-device.
  - Scatter rounds have distinct destinations within each call (race-free
    CCE accumulation); high-degree columns spill to a second virtual level
    summed during readback. Rounds are split into NSUB destination ranges
    with disjoint output APs so sub-calls of one round never serialize.
"""
import sys
import numpy as np

sys.path.insert(0, "/opt/trn_rl_repo")

F = 64           # feature width (STATE == HID == 64)
NCORES = 8


class Plan:
    """Static (compile-time) layout shared by all cores, plus per-core data.

    Node layout: local node l = t*128 + p  (SBUF partition p, tile t).
    Global gather-table row of node (core k, local l) = k*sh + l.
    Scatter destination slot of (level, local l): g_s = level*T + t, range
    j = g_s // Q, s_local = g_s % Q, idx = s_local*128 + p (trash at
    s_local = Q). Buffer pair (A = even s_local, B = odd), group column =
    W//2 * j + s_local//2.
    """

    def __init__(self, n_nodes, edge_index, cap_min=24, row_chunk=32768):
        self.n_nodes = n_nodes
        self.row_chunk = row_chunk
        shard = (n_nodes + NCORES - 1) // NCORES
        sh = ((shard + 127) // 128) * 128
        self.shard, self.sh, self.t = shard, sh, sh // 128
        self.gtbl_rows = ((sh * NCORES + row_chunk - 1) // row_chunk) * row_chunk
        self.n_chunks = self.gtbl_rows // row_chunk

        row = np.asarray(edge_index[0], dtype=np.int64)
        col = np.asarray(edge_index[1], dtype=np.int64)
        deg = np.bincount(col, minlength=n_nodes).astype(np.float64) + 1.0
        self.dis = (1.0 / np.sqrt(deg)).astype(np.float32)

        tpos_row = (row // shard) * sh + (row % shard)
        dst_core = col // shard
        cloc = col % shard

        per_core = []
        maxdeg = 0
        for k in range(NCORES):
            m = dst_core == k
            r_k, c_k = tpos_row[m], cloc[m]
            o = np.argsort(c_k, kind="stable")
            cs = c_k[o]
            if cs.size:
                starts = np.r_[True, cs[1:] != cs[:-1]]
                run_starts = np.flatnonzero(starts)
                rid = np.cumsum(starts) - 1
                occ_s = np.arange(cs.size) - run_starts[rid]
                occ = np.empty_like(occ_s)
                occ[o] = occ_s
                maxdeg = max(maxdeg, int(occ_s.max()) + 1)
            else:
                occ = np.zeros(0, np.int64)
            per_core.append((r_k, c_k, occ))

        self.levels = 2
        self.cap = max(cap_min, (maxdeg + self.levels - 1) // self.levels)
        assert maxdeg <= self.cap * self.levels

        T = self.t
        tot_slots = self.levels * T
        for nsub in (4, 2, 1):
            if tot_slots % nsub == 0:
                self.nsub = nsub
                break
        self.q_slots = tot_slots // self.nsub                 # real slots/range
        self.trash_slots = 2 if self.q_slots % 2 == 0 else 1
        self.w_slots = self.q_slots + self.trash_slots        # even
        assert self.w_slots % 2 == 0
        self.grp_per_range = self.w_slots // 2
        self.agg_groups = self.grp_per_range * self.nsub
        assert (self.w_slots - 1) * 128 + 127 <= 32767

        NS, NC = self.nsub, self.n_chunks
        cnt = np.zeros((NCORES, self.cap, NS, NC), np.int64)
        for k, (r_k, c_k, occ) in enumerate(per_core):
            rnd = occ % self.cap
            lev = occ // self.cap
            g_s = lev * T + (c_k >> 7)
            rng = g_s // self.q_slots
            chk = r_k // row_chunk
            np.add.at(cnt[k], (rnd, rng, chk), 1)
        n_rjc = cnt.max(axis=0)
        n_rjc = np.maximum((n_rjc + 127) // 128 * 128, 128)
        self.n_rjc = n_rjc                                   # [cap, NS, NC]
        self.m_rj = n_rjc.sum(axis=2)
        self.n_r = self.m_rj.sum(axis=1)
        self.max_nr = int(self.n_r.max())
        self.tot_tok = int(self.n_r.sum())

        base_r = np.concatenate([[0], np.cumsum(self.n_r)])[:-1]
        off_rj = np.zeros((self.cap, NS), np.int64)
        off_rjc = np.zeros((self.cap, NS, NC), np.int64)
        for r in range(self.cap):
            o = 0
            for j in range(NS):
                off_rj[r, j] = o
                for c in range(NC):
                    off_rjc[r, j, c] = o
                    o += int(n_rjc[r, j, c])
        self.base_r, self.off_rj, self.off_rjc = base_r, off_rj, off_rjc

        self.gidx = []
        self.sidx = []
        for k, (r_k, c_k, occ) in enumerate(per_core):
            rnd = (occ % self.cap).astype(np.int64)
            lev = (occ // self.cap).astype(np.int64)
            tt = c_k >> 7
            pp = c_k & 127
            g_s = lev * T + tt
            s_local = g_s % self.q_slots
            rng = g_s // self.q_slots
            chk = r_k // row_chunk
            gflat = np.zeros(self.tot_tok, np.int64)
            sflat = np.full(self.tot_tok, self.q_slots * 128, np.int64)
            key = (rnd * NS + rng) * NC + chk
            order = np.argsort(key, kind="stable")
            ks = key[order]
            if ks.size:
                starts = np.r_[True, ks[1:] != ks[:-1]]
                run_starts = np.flatnonzero(starts)
                rid = np.cumsum(starts) - 1
                within = np.arange(ks.size) - run_starts[rid]
                rr = ks // (NS * NC)
                jj = (ks // NC) % NS
                cc = ks % NC
                pos = base_r[rr] + off_rjc[rr, jj, cc] + within
                e = order
                gflat[pos] = r_k[e] - cc * row_chunk
                sflat[pos] = s_local[e] * 128 + pp[e]
            self.gidx.append(self._wrap(gflat))
            self.sidx.append(self._wrap(sflat))

        self.gso = {}
        self.sso = {}
        off = 0
        for r in range(self.cap):
            for j in range(NS):
                for c in range(NC):
                    self.gso[(r, j, c)] = off
                    off += int(n_rjc[r, j, c]) // 16
        self.gslots = off
        off = 0
        for r in range(self.cap):
            for j in range(NS):
                self.sso[(r, j)] = off
                off += int(self.m_rj[r, j]) // 16
        self.sslots = off

    @staticmethod
    def _wrap(idx):
        n = idx.size
        a = idx.astype(np.int16).reshape(n // 16, 16).T
        return np.ascontiguousarray(np.tile(a, (8, 1)))

    def core_inputs(self, k, x, W1, b1, W2, b2, W3, b3):
        sh, shard, t = self.sh, self.shard, self.t
        xs = np.zeros((sh, F), np.float32)
        lo, hi = k * shard, min((k + 1) * shard, self.n_nodes)
        xs[: hi - lo] = x[lo:hi]
        ds = np.zeros(sh, np.float32)
        ds[: hi - lo] = self.dis[lo:hi]
        x_dev = xs.reshape(t, 128, F).transpose(1, 0, 2)   # node l = t*128+p
        d_dev = ds.reshape(t, 128).T
        return {
            "x": np.ascontiguousarray(x_dev.reshape(128, t * F)),
            "dis": np.ascontiguousarray(d_dev),
            "gidx": self.gidx[k],
            "sidx": self.sidx[k],
            "W1": np.asarray(W1, np.float32),
            "b1": np.asarray(b1, np.float32).reshape(F, 1),
            "W2": np.asarray(W2, np.float32),
            "b2": np.asarray(b2, np.float32).reshape(F, 1),
            "W3": np.asarray(W3, np.float32).reshape(F, 1),
            "b3": np.asarray(b3, np.float32).reshape(1, 1),
        }

    def assemble(self, outs):
        """outs: per core {'out': [sh]} with flat index == node local id."""
        res = np.zeros((self.n_nodes, 1), np.float32)
        for k in range(NCORES):
            o = np.asarray(outs[k]["out"]).reshape(self.sh)
            lo = k * self.shard
            hi = min(lo + self.shard, self.n_nodes)
            res[lo:hi, 0] = o[: hi - lo]
        return res


def build(plan, n_layers=3):
    import concourse.bacc as bacc
    import concourse.mybir as mybir
    import concourse.tile as tile
    from concourse.masks import make_identity

    f32 = mybir.dt.float32
    i16 = mybir.dt.int16
    T, SH = plan.t, plan.sh
    CAP, NC, NS = plan.cap, plan.n_chunks, plan.nsub
    MAXNR = plan.max_nr
    GPR = plan.grp_per_range

    nc = bacc.Bacc("TRN2", target_bir_lowering=False, debug=False,
                   num_devices=NCORES, num_swdge_queues=4)

    x_t = nc.dram_tensor("x", [128, T * F], f32, kind="ExternalInput")
    dis_t = nc.dram_tensor("dis", [128, T], f32, kind="ExternalInput")
    gidx_t = nc.dram_tensor("gidx", [128, plan.gslots], i16, kind="ExternalInput")
    sidx_t = nc.dram_tensor("sidx", [128, plan.sslots], i16, kind="ExternalInput")
    Ws = {}
    for nm, shape in [("W1", [F, F]), ("b1", [F, 1]), ("W2", [F, F]),
                      ("b2", [F, 1]), ("W3", [F, 1]), ("b3", [1, 1])]:
        Ws[nm] = nc.dram_tensor(nm, shape, f32, kind="ExternalInput")
    out_t = nc.dram_tensor("out", [SH], f32, kind="ExternalOutput")

    g_dram = nc.dram_tensor("g_bounce", [SH * F], f32, kind="Internal")
    G = nc.dram_tensor("G_table", [plan.gtbl_rows, F], f32, kind="Internal",
                       addr_space="Shared")
    rg = [list(range(NCORES))]

    with tile.TileContext(nc) as tc:
        with tc.tile_pool(name="const", bufs=1) as cpool, \
             tc.tile_pool(name="state", bufs=1) as spool, \
             tc.tile_pool(name="agg", bufs=1) as apool, \
             tc.tile_pool(name="msg", bufs=2) as mpool, \
             tc.tile_pool(name="idx", bufs=2) as ipool, \
             tc.tile_pool(name="fm", bufs=2) as fpool, \
             tc.tile_pool(name="psum", bufs=2, space="PSUM") as ppool:

            ident = cpool.tile([128, 128], f32)
            make_identity(nc, ident[:])
            dis_s = cpool.tile([128, T], f32)
            nc.sync.dma_start(dis_s[:], dis_t[:])
            wsb = {}
            for nm in ("W1", "W2", "W3", "b1", "b2", "b3"):
                wsb[nm] = cpool.tile(list(Ws[nm].shape), f32, name=f"sb_{nm}")
                nc.sync.dma_start(wsb[nm][:], Ws[nm][:])

            dis_b = dis_s[:].unsqueeze(-1).broadcast_to([128, T, F])

            aggA = apool.tile([128, plan.agg_groups, F], f32, tag="aggA")
            aggB = apool.tile([128, plan.agg_groups, F], f32, tag="aggB")

            g = spool.tile([128, T, F], f32, tag="g")
            tmp = spool.tile([128, T, F], f32, tag="acc")
            nc.sync.dma_start(tmp[:], x_t[:].rearrange("p (t f) -> p t f", f=F))
            nc.vector.tensor_tensor(g[:], tmp[:], dis_b, mybir.AluOpType.mult)

            # strided view: DRAM row l = t*128+p  <->  g[p, t, :]
            g_dram_v = g_dram[:].rearrange("(t p f) -> p t f", p=128, f=F)

            def agg_src(lev, t):
                g_s = lev * T + t
                j, s = divmod(g_s, plan.q_slots)
                buf = aggA if s % 2 == 0 else aggB
                return buf, GPR * j + s // 2

            for layer in range(n_layers):
                nc.sync.dma_start(g_dram_v, g[:])
                nc.gpsimd.collective_compute(
                    "AllGather", mybir.AluOpType.bypass,
                    replica_groups=rg,
                    ins=[g_dram[:]],
                    outs=[G[0:SH * NCORES, :].rearrange("r f -> (r f)")],
                )
                nc.vector.memset(aggA[:], 0.0)
                nc.vector.memset(aggB[:], 0.0)

                for r in range(CAP):
                    n_r = int(plan.n_r[r])
                    gi = ipool.tile([128, MAXNR // 16], i16, tag="gi")
                    si = ipool.tile([128, MAXNR // 16], i16, tag="si")
                    g0 = plan.gso[(r, 0, 0)]
                    s0 = plan.sso[(r, 0)]
                    nc.sync.dma_start(gi[:, : n_r // 16],
                                      gidx_t[:, g0:g0 + n_r // 16])
                    nc.sync.dma_start(si[:, : n_r // 16],
                                      sidx_t[:, s0:s0 + n_r // 16])
                    buf = mpool.tile([128, MAXNR // 128, F], f32, tag="msg")
                    for j in range(NS):
                        oj = int(plan.off_rj[r, j])
                        for c in range(NC):
                            n_rjc = int(plan.n_rjc[r, j, c])
                            oc = int(plan.off_rjc[r, j, c])
                            go = plan.gso[(r, j, c)] - g0
                            nc.gpsimd.dma_gather(
                                buf[:, oc // 128:(oc + n_rjc) // 128, :],
                                G[c * plan.row_chunk:(c + 1) * plan.row_chunk, :],
                                gi[:, go:go + n_rjc // 16],
                                n_rjc, n_rjc, F,
                                queue_num=2 * (j % 2),
                                single_packet=bool(n_rjc <= 1024))
                        m_rj = int(plan.m_rj[r, j])
                        so = plan.sso[(r, j)] - s0
                        nc.gpsimd.dma_scatter_add(
                            aggA[:, GPR * j:GPR * (j + 1), :],
                            buf[:, oj // 128:(oj + m_rj) // 128, :],
                            si[:, so:so + m_rj // 16],
                            m_rj, m_rj, F,
                            sbuf_tokens_per_rank=128,
                            parity_reg=0,
                            out_ap_other=aggB[:, GPR * j:GPR * (j + 1), :],
                            queue_num=1 + 2 * (j % 2))

                acc = spool.tile([128, T, F], f32, tag="acc")
                for t in range(T):
                    b0, g0i = agg_src(0, t)
                    b1v, g1i = agg_src(1, t)
                    nc.vector.tensor_tensor(acc[:, t, :], b0[:, g0i, :],
                                            b1v[:, g1i, :], mybir.AluOpType.add)
                nc.vector.tensor_tensor(acc[:], acc[:], g[:], mybir.AluOpType.add)
                nc.vector.tensor_tensor(acc[:], acc[:], dis_b, mybir.AluOpType.mult)

                if layer < n_layers - 1:
                    W, b = wsb[f"W{layer + 1}"], wsb[f"b{layer + 1}"]
                    g2 = spool.tile([128, T, F], f32, tag="g")
                    for t0 in range(0, T, 4):
                        nt = min(4, T - t0)
                        fm = fpool.tile([F, 4 * 128], f32, tag="fm")
                        for j in range(nt):
                            pt = ppool.tile([F, 128], f32, space="PSUM", tag="pt")
                            nc.tensor.transpose(pt[:], acc[:, t0 + j, :], ident[:])
                            nc.vector.tensor_copy(fm[:, j * 128:(j + 1) * 128], pt[:])
                        mm = ppool.tile([F, 4 * 128], f32, space="PSUM", tag="mm")
                        nc.tensor.matmul(mm[:, : nt * 128], W[:], fm[:, : nt * 128],
                                         start=True, stop=True)
                        hfm = fpool.tile([F, 4 * 128], f32, tag="hfm")
                        nc.scalar.activation(hfm[:, : nt * 128], mm[:, : nt * 128],
                                             mybir.ActivationFunctionType.Relu,
                                             bias=b[:, :1])
                        for j in range(nt):
                            pt2 = ppool.tile([128, F], f32, space="PSUM", tag="pt2")
                            nc.tensor.transpose(
                                pt2[:], hfm[:, j * 128:(j + 1) * 128], ident[:F, :F])
                            nc.vector.tensor_scalar_mul(
                                g2[:, t0 + j, :], pt2[:], dis_s[:, t0 + j:t0 + j + 1])
                    g = g2
                else:
                    W3, b3 = wsb["W3"], wsb["b3"]
                    for t0 in range(0, T, 4):
                        nt = min(4, T - t0)
                        fm = fpool.tile([F, 4 * 128], f32, tag="fm")
                        for j in range(nt):
                            pt = ppool.tile([F, 128], f32, space="PSUM", tag="pt")
                            nc.tensor.transpose(pt[:], acc[:, t0 + j, :], ident[:])
                            nc.vector.tensor_copy(fm[:, j * 128:(j + 1) * 128], pt[:])
                        mm3 = ppool.tile([1, 4 * 128], f32, space="PSUM", tag="mm")
                        nc.tensor.matmul(mm3[:, : nt * 128], W3[:], fm[:, : nt * 128],
                                         start=True, stop=True)
                        ofm = fpool.tile([1, 4 * 128], f32, tag="ofm")
                        nc.vector.tensor_scalar_add(
                            ofm[:, : nt * 128], mm3[:, : nt * 128], b3[:, :1])
                        nc.sync.dma_start(
                            out_t[t0 * 128:(t0 + nt) * 128]
                            .rearrange("(a x) -> a x", a=1),
                            ofm[:, : nt * 128])

    nc.compile()
    return nc


def kernel(**inputs):
    from concourse import bass2jax

    x = np.asarray(inputs["x"], np.float32)
    edge_index = np.asarray(inputs["edge_index"])
    plan = Plan(x.shape[0], edge_index)
    nc = build(plan)
    in_maps = [plan.core_inputs(k, x, inputs["W1"], inputs["b1"], inputs["W2"],
                                inputs["b2"], inputs["W3"], inputs["b3"])
               for k in range(NCORES)]
    results = bass2jax.run_bass_via_pjrt(nc, in_maps, n_cores=NCORES)
    return plan.assemble(results)



# revision 3
# speedup vs baseline: 1.0666x; 1.0666x over previous
"""3-layer GCN on Trainium2, 8 NeuronCores — matmul-aggregation design.

Strategy (graph/data parallel):
  - Nodes block-partitioned across 8 cores (dst-sharded); weights replicated.
  - Per layer each core computes g = dis * h for its shard; shards are
    AllGathered into a shared fp16 HBM table G (rows padded to 256B so
    dma_gather's 256B-element constraint is met).
  - Aggregation: edge tokens are sorted by destination tile (128 dst nodes);
    each token's source row is dma_gathered into a token-major SBUF buffer.
    Per 128-token group, a one-hot selection matrix S (S[tok, dst] =
    (dlo[tok] == dst)) is built on the DVE from a precomputed per-token
    destination id, and the PE accumulates psum[dst, f] += S^T @ msg into
    PSUM. No scatter-add is needed anywhere.
  - Self-loops are extra tokens; padding tokens use dlo=200 so their S row
    is all-zero (gathered garbage contributes nothing).
  - The G table is chunked so gather indices fit int16: chunk q holds
    quarter q of every core's shard, and is filled by its own AllGather so
    gathers of chunk q can start before later chunks arrive.
"""
import sys
import numpy as np

sys.path.insert(0, "/opt/trn_rl_repo")

F = 64
NCORES = 8
SBS = 5            # dst tiles per superblock (gather/matmul pipeline unit)
PAD_DLO = 200.0    # one-hot miss -> padding tokens contribute nothing


class Plan:
    def __init__(self, n_nodes, edge_index):
        self.n_nodes = n_nodes
        shard = (n_nodes + NCORES - 1) // NCORES
        sh = ((shard + 127) // 128) * 128
        self.shard, self.sh, self.t = shard, sh, sh // 128
        T = self.t

        # quarters of each shard (tile-aligned) -> 4 gather chunks
        qt = [0, 25, 50, 75, T]
        self.qt = qt
        qlo = np.array([q * 128 for q in qt[:-1]])
        qrows = np.array([(qt[i + 1] - qt[i]) * 128 for i in range(4)])
        chunk_base = np.concatenate([[0], np.cumsum(qrows * NCORES)])[:4]
        self.qrows, self.chunk_base = qrows, chunk_base
        self.gtbl_rows = int((qrows * NCORES).sum())
        assert self.gtbl_rows == sh * NCORES
        assert all(qrows * NCORES <= 32767)

        row = np.asarray(edge_index[0], dtype=np.int64)
        col = np.asarray(edge_index[1], dtype=np.int64)
        deg = np.bincount(col, minlength=n_nodes).astype(np.float64) + 1.0
        self.dis = (1.0 / np.sqrt(deg)).astype(np.float32)

        def table_pos(src_core, src_l):
            srq = np.minimum(src_l >> 7, T - 1) // 25
            srq = np.minimum(srq, 3)
            inchunk = src_core * qrows[srq] + (src_l - qlo[srq])
            return srq, inchunk

        src_core = row // shard
        src_l = row % shard
        dst_core = col // shard
        cloc = col % shard
        ch_e, ic_e = table_pos(src_core, src_l)

        NSB = (T + SBS - 1) // SBS
        self.nsb = NSB

        # per-core token sets (with self loops)
        per_core = []
        for k in range(NCORES):
            m = dst_core == k
            sc = np.full(shard, k, np.int64)
            sl = np.arange(shard, dtype=np.int64)
            ch_s, ic_s = table_pos(sc, sl)
            ch_k = np.concatenate([ch_e[m], ch_s])
            ic_k = np.concatenate([ic_e[m], ic_s])
            c_k = np.concatenate([cloc[m], sl])
            per_core.append((ch_k, ic_k, c_k))

        # uniform run sizes: n[sb, ch, t] = roundup(max_k count, 128)
        cnt = np.zeros((NCORES, NSB, 4, T), np.int64)
        for k, (ch_k, ic_k, c_k) in enumerate(per_core):
            t_k = c_k >> 7
            sb_k = t_k // SBS
            np.add.at(cnt[k], (sb_k, ch_k, t_k), 1)
        mx = cnt.max(axis=0)
        n_sbct = np.where(mx > 0, ((mx + 127) // 128) * 128, 0)
        self.n_sbct = n_sbct

        # offsets in schedule order (sb, ch, t)
        off_sbct = np.zeros((NSB, 4, T), np.int64)
        self.sb_off = np.zeros(NSB + 1, np.int64)
        self.n_sbc = np.zeros((NSB, 4), np.int64)
        self.off_sbc = np.zeros((NSB, 4), np.int64)
        o = 0
        for sb in range(NSB):
            self.sb_off[sb] = o
            t0, t1 = sb * SBS, min((sb + 1) * SBS, T)
            for ch in range(4):
                self.off_sbc[sb, ch] = o
                for t in range(t0, t1):
                    off_sbct[sb, ch, t] = o
                    o += int(n_sbct[sb, ch, t])
                self.n_sbc[sb, ch] = o - self.off_sbc[sb, ch]
        self.sb_off[NSB] = o
        self.tot = o
        self.tok_sb_max = int((self.sb_off[1:] - self.sb_off[:-1]).max())
        self.cols_max = self.tok_sb_max // 128

        # per-tile matmul groups: column indices relative to the sb base
        self.groups = []          # [t] -> list of sb-relative col indices
        for t in range(T):
            sb = t // SBS
            base = self.sb_off[sb]
            cols = []
            for ch in range(4):
                go = (off_sbct[sb, ch, t] - base) // 128
                for i in range(int(n_sbct[sb, ch, t]) // 128):
                    cols.append(int(go + i))
            self.groups.append(cols)

        # per-core gather index + dlo tables
        self.gidx = []
        self.dlo = []
        for k, (ch_k, ic_k, c_k) in enumerate(per_core):
            t_k = c_k >> 7
            sb_k = t_k // SBS
            key = (sb_k * 4 + ch_k) * T + t_k
            order = np.argsort(key, kind="stable")
            ks = key[order]
            gflat = np.zeros(self.tot, np.int64)
            dflat = np.full(self.tot, PAD_DLO, np.float32)
            if ks.size:
                starts = np.r_[True, ks[1:] != ks[:-1]]
                run_starts = np.flatnonzero(starts)
                rid = np.cumsum(starts) - 1
                within = np.arange(ks.size) - run_starts[rid]
                sbv = ks // (4 * T)
                chv = (ks // T) % 4
                tv = ks % T
                pos = off_sbct[sbv, chv, tv] + within
                e = order
                gflat[pos] = ic_k[e]
                dflat[pos] = (c_k[e] & 127).astype(np.float32)
            self.gidx.append(self._wrap16(gflat))
            self.dlo.append(np.ascontiguousarray(
                dflat.reshape(self.tot // 128, 128).T.astype(np.float16)))
            self._gflat_dbg = getattr(self, "_gflat_dbg", [])
            self._gflat_dbg.append(gflat)
            self._dflat_dbg = getattr(self, "_dflat_dbg", [])
            self._dflat_dbg.append(dflat)

        # debug: map absolute token-col -> (tile, chunk); -1 where unused
        self.colmap = np.full(self.tot // 128, -1, np.int64)
        self.colch = np.full(self.tot // 128, -1, np.int64)
        for t in range(T):
            sb = t // SBS
            for ch in range(4):
                go = off_sbct[sb, ch, t]
                for i in range(int(n_sbct[sb, ch, t]) // 128):
                    self.colmap[go // 128 + i] = t
                    self.colch[go // 128 + i] = ch

    @staticmethod
    def _wrap16(idx):
        n = idx.size
        a = idx.astype(np.int16).reshape(n // 16, 16).T
        return np.ascontiguousarray(np.tile(a, (8, 1)))

    def core_inputs(self, k, x, W1, b1, W2, b2, W3, b3):
        sh, shard, t = self.sh, self.shard, self.t
        xs = np.zeros((sh, F), np.float32)
        lo, hi = k * shard, min((k + 1) * shard, self.n_nodes)
        xs[: hi - lo] = x[lo:hi]
        ds = np.zeros(sh, np.float32)
        ds[: hi - lo] = self.dis[lo:hi]
        x_dev = xs.reshape(t, 128, F).transpose(1, 0, 2)
        d_dev = ds.reshape(t, 128).T
        iota = np.broadcast_to(np.arange(128, dtype=np.float16), (128, 128))
        return {
            "x": np.ascontiguousarray(x_dev.reshape(128, t * F)),
            "dis": np.ascontiguousarray(d_dev),
            "gidx": self.gidx[k],
            "dlo": self.dlo[k],
            "iota": np.ascontiguousarray(iota),
            "W1": np.asarray(W1, np.float16),
            "b1": np.asarray(b1, np.float32).reshape(F, 1),
            "W2": np.asarray(W2, np.float16),
            "b2": np.asarray(b2, np.float32).reshape(F, 1),
            "W3": np.asarray(W3, np.float16).reshape(F, 1),
            "b3": np.asarray(b3, np.float32).reshape(1, 1),
        }

    def assemble(self, outs):
        res = np.zeros((self.n_nodes, 1), np.float32)
        for k in range(NCORES):
            o = np.asarray(outs[k]["out"]).reshape(self.sh)
            lo = k * self.shard
            hi = min(lo + self.shard, self.n_nodes)
            res[lo:hi, 0] = o[: hi - lo]
        return res


def build(plan, n_layers=3):
    import concourse.bacc as bacc
    import concourse.mybir as mybir
    import concourse.tile as tile
    from concourse.masks import make_identity

    f32 = mybir.dt.float32
    f16 = mybir.dt.float16
    i16 = mybir.dt.int16
    T, SH, NSB = plan.t, plan.sh, plan.nsb
    COLS = plan.cols_max

    nc = bacc.Bacc("TRN2", target_bir_lowering=False, debug=False,
                   num_devices=NCORES, num_swdge_queues=4)

    x_t = nc.dram_tensor("x", [128, T * F], f32, kind="ExternalInput")
    dis_t = nc.dram_tensor("dis", [128, T], f32, kind="ExternalInput")
    gidx_t = nc.dram_tensor("gidx", [128, plan.tot // 16], i16, kind="ExternalInput")
    dlo_t = nc.dram_tensor("dlo", [128, plan.tot // 128], f16, kind="ExternalInput")
    iota_t = nc.dram_tensor("iota", [128, 128], f16, kind="ExternalInput")
    Ws = {}
    for nm, shape, dt in [("W1", [F, F], f16), ("b1", [F, 1], f32),
                          ("W2", [F, F], f16), ("b2", [F, 1], f32),
                          ("W3", [F, 1], f16), ("b3", [1, 1], f32)]:
        Ws[nm] = nc.dram_tensor(nm, shape, dt, kind="ExternalInput")
    out_t = nc.dram_tensor("out", [SH], f32, kind="ExternalOutput")

    g_dram = nc.dram_tensor("g_bounce", [SH * 128], f16, kind="Internal")
    # double-buffered across layers: layer L+1's AllGather must not overwrite
    # the table while layer L's tail gathers still read it (cross-layer WAR
    # on DRAM is not tracked by the scheduler)
    G_bufs = [nc.dram_tensor(f"G_table{i}", [plan.gtbl_rows, 128], f16,
                             kind="Internal", addr_space="Shared")
              for i in range(2)]
    rg = [list(range(NCORES))]
    qt, qrows, chunk_base = plan.qt, plan.qrows, plan.chunk_base

    with tile.TileContext(nc) as tc:
        with tc.tile_pool(name="const", bufs=1) as cpool, \
             tc.tile_pool(name="gst", bufs=1) as gpool, \
             tc.tile_pool(name="msg", bufs=2) as mpool, \
             tc.tile_pool(name="sel", bufs=2) as spool, \
             tc.tile_pool(name="idx", bufs=2) as ipool, \
             tc.tile_pool(name="acc", bufs=2) as apool, \
             tc.tile_pool(name="fm", bufs=2) as fpool, \
             tc.tile_pool(name="pagg", bufs=2, space="PSUM") as pagg, \
             tc.tile_pool(name="ptp", bufs=2, space="PSUM") as ptp, \
             tc.tile_pool(name="pt2", bufs=2, space="PSUM") as pt2p, \
             tc.tile_pool(name="pmm", bufs=2, space="PSUM") as pmm:

            ident = cpool.tile([128, 128], f32)
            make_identity(nc, ident[:])
            ident16 = cpool.tile([128, 128], f16, name="ident16")
            make_identity(nc, ident16[:])
            dis_s = cpool.tile([128, T], f32)
            nc.sync.dma_start(dis_s[:], dis_t[:])
            iota_s = cpool.tile([128, 128], f16, name="iota_s")
            nc.sync.dma_start(iota_s[:], iota_t[:])
            dlo_s = cpool.tile([128, plan.tot // 128], f16, name="dlo_s")
            nc.sync.dma_start(dlo_s[:], dlo_t[:])
            wsb = {}
            for nm in ("W1", "W2", "W3", "b1", "b2", "b3"):
                wsb[nm] = cpool.tile(list(Ws[nm].shape), Ws[nm].dtype,
                                     name=f"sb_{nm}")
                nc.sync.dma_start(wsb[nm][:], Ws[nm][:])

            dis_b = dis_s[:].unsqueeze(-1).broadcast_to([128, T, F])

            xin = mpool.tile([128, T, F], f32, tag="msg")
            nc.sync.dma_start(xin[:], x_t[:].rearrange("p (t f) -> p t f", f=F))
            g = gpool.tile([128, T, 128], f16, tag="g0")
            nc.vector.tensor_tensor(g[:, :, :F], xin[:], dis_b,
                                    mybir.AluOpType.mult)

            g_dram_v = g_dram[:].rearrange("(t p f) -> p t f", p=128, f=128)

            for layer in range(n_layers):
                G = G_bufs[layer % 2]
                # publish g quarters; chunk q of G is its own AllGather
                for q in range(4):
                    t0, t1 = qt[q], qt[q + 1]
                    nc.sync.dma_start(g_dram_v[:, t0:t1, :], g[:, t0:t1, :])
                    nc.gpsimd.collective_compute(
                        "AllGather", mybir.AluOpType.bypass,
                        replica_groups=rg,
                        ins=[g_dram[t0 * 128 * 128:t1 * 128 * 128]],
                        outs=[G[int(chunk_base[q]):
                                int(chunk_base[q]) + NCORES * int(qrows[q]), :]
                              .rearrange("r f -> (r f)")],
                    )

                if layer < n_layers - 1:
                    g_next = gpool.tile([128, T, 128], f16,
                                        tag=f"g{(layer + 1) % 2}")

                acc4 = None
                for sb in range(NSB):
                    base = int(plan.sb_off[sb])
                    tok_sb = int(plan.sb_off[sb + 1]) - base
                    cols_sb = tok_sb // 128
                    gi = ipool.tile([128, plan.tok_sb_max // 16], i16, tag="gi")
                    nc.sync.dma_start(
                        gi[:, : tok_sb // 16],
                        gidx_t[:, base // 16:(base + tok_sb) // 16])
                    msgb = mpool.tile([128, COLS, 128], f16, tag="msg")
                    for ch in range(4):
                        n = int(plan.n_sbc[sb, ch])
                        if n == 0:
                            continue
                        co = (int(plan.off_sbc[sb, ch]) - base) // 128
                        go = (int(plan.off_sbc[sb, ch]) - base) // 16
                        nc.gpsimd.dma_gather(
                            msgb[:, co:co + n // 128, :],
                            G[int(chunk_base[ch]):
                              int(chunk_base[ch]) + NCORES * int(qrows[ch]), :],
                            gi[:, go:go + n // 16],
                            n, n, 128,
                            queue_num=ch,
                            single_packet=bool(n <= 1024))
                    S = spool.tile([128, COLS, 128], f16, tag="S")
                    nc.vector.tensor_tensor(
                        S[:, :cols_sb, :],
                        dlo_s[:, base // 128:base // 128 + cols_sb]
                        .unsqueeze(-1).broadcast_to([128, cols_sb, 128]),
                        iota_s[:].unsqueeze(1).broadcast_to([128, cols_sb, 128]),
                        mybir.AluOpType.is_equal)

                    for t in range(sb * SBS, min((sb + 1) * SBS, T)):
                        cols = plan.groups[t]
                        pa = pagg.tile([128, F], f32, tag="agg")
                        for i, cidx in enumerate(cols):
                            nc.tensor.matmul(
                                pa[:], S[:, cidx, :], msgb[:, cidx, :F],
                                start=(i == 0), stop=(i == len(cols) - 1))
                        j = t % 4
                        if j == 0:
                            acc4 = apool.tile([128, 4, F], f32, tag="acc4")
                        nc.vector.tensor_scalar_mul(
                            acc4[:, j, :], pa[:], dis_s[:, t:t + 1])
                        if j == 3 or t == T - 1:
                            t0 = t - j
                            nt = j + 1
                            fm = fpool.tile([F, 4 * 128], f16, tag="fm")
                            for j2 in range(nt):
                                pt = ptp.tile([F, 128], f32, tag="pt")
                                nc.tensor.transpose(pt[:], acc4[:, j2, :],
                                                    ident[:])
                                nc.vector.tensor_copy(
                                    fm[:, j2 * 128:(j2 + 1) * 128], pt[:])
                            if layer < n_layers - 1:
                                W, b = wsb[f"W{layer + 1}"], wsb[f"b{layer + 1}"]
                                mm = pmm.tile([F, 4 * 128], f32, tag="mm")
                                nc.tensor.matmul(mm[:, : nt * 128], W[:],
                                                 fm[:, : nt * 128],
                                                 start=True, stop=True)
                                hfm = fpool.tile([F, 4 * 128], f16, tag="hfm")
                                nc.scalar.activation(
                                    hfm[:, : nt * 128], mm[:, : nt * 128],
                                    mybir.ActivationFunctionType.Relu,
                                    bias=b[:, :1])
                                for j2 in range(nt):
                                    p2 = pt2p.tile([128, F], f16, tag="pt2")
                                    nc.tensor.transpose(
                                        p2[:], hfm[:, j2 * 128:(j2 + 1) * 128],
                                        ident16[:F, :F])
                                    nc.vector.tensor_scalar_mul(
                                        g_next[:, t0 + j2, :F], p2[:],
                                        dis_s[:, t0 + j2:t0 + j2 + 1])
                            else:
                                W3, b3 = wsb["W3"], wsb["b3"]
                                mm3 = pmm.tile([1, 4 * 128], f32, tag="mm")
                                nc.tensor.matmul(mm3[:, : nt * 128], W3[:],
                                                 fm[:, : nt * 128],
                                                 start=True, stop=True)
                                ofm = fpool.tile([1, 4 * 128], f32, tag="ofm")
                                nc.vector.tensor_scalar_add(
                                    ofm[:, : nt * 128], mm3[:, : nt * 128],
                                    b3[:, :1])
                                nc.sync.dma_start(
                                    out_t[t0 * 128:(t0 + nt) * 128]
                                    .rearrange("(a x) -> a x", a=1),
                                    ofm[:, : nt * 128])
                if layer < n_layers - 1:
                    g = g_next

    nc.compile()
    return nc


def kernel(**inputs):
    import sys
    sys.path.insert(0, "/opt/trn_rl_repo")
    from concourse import bass2jax

    x = np.asarray(inputs["x"], np.float32)
    edge_index = np.asarray(inputs["edge_index"])
    plan = Plan(x.shape[0], edge_index)
    nc = build(plan)
    in_maps = [plan.core_inputs(k, x, inputs["W1"], inputs["b1"], inputs["W2"],
                                inputs["b2"], inputs["W3"], inputs["b3"])
               for k in range(NCORES)]
    results = bass2jax.run_bass_via_pjrt(nc, in_maps, n_cores=NCORES)
    return plan.assemble(results)


# revision 5
# speedup vs baseline: 1.0789x; 1.0116x over previous
"""3-layer GCN on Trainium2, 8 NeuronCores — matmul-aggregation design.

Strategy (graph/data parallel):
  - Nodes block-partitioned across 8 cores (dst-sharded); weights replicated.
  - Per layer each core computes g = dis * h for its shard; shards are
    AllGathered into a shared fp16 HBM table G (rows padded to 256B so
    dma_gather's 256B-element constraint is met).
  - Aggregation: edge tokens are sorted by destination tile (128 dst nodes);
    each token's source row is dma_gathered into a token-major SBUF buffer.
    Per 128-token group, a one-hot selection matrix S (S[tok, dst] =
    (dlo[tok] == dst)) is built on the DVE from a precomputed per-token
    destination id, and the PE accumulates psum[dst, f] += S^T @ msg into
    PSUM. No scatter-add is needed anywhere.
  - Self-loops are extra tokens; padding tokens use dlo=200 so their S row
    is all-zero (gathered garbage contributes nothing).
  - The G table is chunked so gather indices fit int16: chunk q holds
    quarter q of every core's shard, and is filled by its own AllGather so
    gathers of chunk q can start before later chunks arrive.
"""
import sys
import numpy as np

sys.path.insert(0, "/opt/trn_rl_repo")

F = 64
NCORES = 8
SBS = 5            # dst tiles per superblock (gather/matmul pipeline unit)
PAD_DLO = 200.0    # one-hot miss -> padding tokens contribute nothing


class Plan:
    def __init__(self, n_nodes, edge_index):
        self.n_nodes = n_nodes
        shard = (n_nodes + NCORES - 1) // NCORES
        sh = ((shard + 127) // 128) * 128
        self.shard, self.sh, self.t = shard, sh, sh // 128
        T = self.t

        # quarters of each shard (tile-aligned) -> 4 gather chunks
        qt = [0, 25, 50, 75, T]
        self.qt = qt
        qlo = np.array([q * 128 for q in qt[:-1]])
        qrows = np.array([(qt[i + 1] - qt[i]) * 128 for i in range(4)])
        chunk_base = np.concatenate([[0], np.cumsum(qrows * NCORES)])[:4]
        self.qrows, self.chunk_base = qrows, chunk_base
        self.gtbl_rows = int((qrows * NCORES).sum())
        assert self.gtbl_rows == sh * NCORES
        assert all(qrows * NCORES <= 32767)

        row = np.asarray(edge_index[0], dtype=np.int64)
        col = np.asarray(edge_index[1], dtype=np.int64)
        deg = np.bincount(col, minlength=n_nodes).astype(np.float64) + 1.0
        self.dis = (1.0 / np.sqrt(deg)).astype(np.float32)

        def table_pos(src_core, src_l):
            srq = np.minimum(src_l >> 7, T - 1) // 25
            srq = np.minimum(srq, 3)
            inchunk = src_core * qrows[srq] + (src_l - qlo[srq])
            return srq, inchunk

        src_core = row // shard
        src_l = row % shard
        dst_core = col // shard
        cloc = col % shard
        ch_e, ic_e = table_pos(src_core, src_l)

        NSB = (T + SBS - 1) // SBS
        self.nsb = NSB

        # per-core token sets (with self loops)
        per_core = []
        for k in range(NCORES):
            m = dst_core == k
            sc = np.full(shard, k, np.int64)
            sl = np.arange(shard, dtype=np.int64)
            ch_s, ic_s = table_pos(sc, sl)
            ch_k = np.concatenate([ch_e[m], ch_s])
            ic_k = np.concatenate([ic_e[m], ic_s])
            c_k = np.concatenate([cloc[m], sl])
            per_core.append((ch_k, ic_k, c_k))

        # uniform run sizes: n[sb, ch, t] = roundup(max_k count, 128)
        cnt = np.zeros((NCORES, NSB, 4, T), np.int64)
        for k, (ch_k, ic_k, c_k) in enumerate(per_core):
            t_k = c_k >> 7
            sb_k = t_k // SBS
            np.add.at(cnt[k], (sb_k, ch_k, t_k), 1)
        mx = cnt.max(axis=0)
        n_sbct = np.where(mx > 0, ((mx + 127) // 128) * 128, 0)
        self.n_sbct = n_sbct

        # offsets in schedule order (sb, ch, t)
        off_sbct = np.zeros((NSB, 4, T), np.int64)
        self.sb_off = np.zeros(NSB + 1, np.int64)
        self.n_sbc = np.zeros((NSB, 4), np.int64)
        self.off_sbc = np.zeros((NSB, 4), np.int64)
        o = 0
        for sb in range(NSB):
            self.sb_off[sb] = o
            t0, t1 = sb * SBS, min((sb + 1) * SBS, T)
            for ch in range(4):
                self.off_sbc[sb, ch] = o
                for t in range(t0, t1):
                    off_sbct[sb, ch, t] = o
                    o += int(n_sbct[sb, ch, t])
                self.n_sbc[sb, ch] = o - self.off_sbc[sb, ch]
        self.sb_off[NSB] = o
        self.tot = o
        self.tok_sb_max = int((self.sb_off[1:] - self.sb_off[:-1]).max())
        self.cols_max = self.tok_sb_max // 128

        # per-tile matmul groups: column indices relative to the sb base
        self.groups = []          # [t] -> list of sb-relative col indices
        for t in range(T):
            sb = t // SBS
            base = self.sb_off[sb]
            cols = []
            for ch in range(4):
                go = (off_sbct[sb, ch, t] - base) // 128
                for i in range(int(n_sbct[sb, ch, t]) // 128):
                    cols.append(int(go + i))
            self.groups.append(cols)

        # per-core gather index + dlo tables
        self.gidx = []
        self.dlo = []
        for k, (ch_k, ic_k, c_k) in enumerate(per_core):
            t_k = c_k >> 7
            sb_k = t_k // SBS
            key = (sb_k * 4 + ch_k) * T + t_k
            order = np.argsort(key, kind="stable")
            ks = key[order]
            gflat = np.zeros(self.tot, np.int64)
            dflat = np.full(self.tot, PAD_DLO, np.float32)
            if ks.size:
                starts = np.r_[True, ks[1:] != ks[:-1]]
                run_starts = np.flatnonzero(starts)
                rid = np.cumsum(starts) - 1
                within = np.arange(ks.size) - run_starts[rid]
                sbv = ks // (4 * T)
                chv = (ks // T) % 4
                tv = ks % T
                pos = off_sbct[sbv, chv, tv] + within
                e = order
                gflat[pos] = ic_k[e]
                dflat[pos] = (c_k[e] & 127).astype(np.float32)
            self.gidx.append(self._wrap16(gflat))
            self.dlo.append(np.ascontiguousarray(
                dflat.reshape(self.tot // 128, 128).T.astype(np.float16)))
            self._gflat_dbg = getattr(self, "_gflat_dbg", [])
            self._gflat_dbg.append(gflat)
            self._dflat_dbg = getattr(self, "_dflat_dbg", [])
            self._dflat_dbg.append(dflat)

        # debug: map absolute token-col -> (tile, chunk); -1 where unused
        self.colmap = np.full(self.tot // 128, -1, np.int64)
        self.colch = np.full(self.tot // 128, -1, np.int64)
        for t in range(T):
            sb = t // SBS
            for ch in range(4):
                go = off_sbct[sb, ch, t]
                for i in range(int(n_sbct[sb, ch, t]) // 128):
                    self.colmap[go // 128 + i] = t
                    self.colch[go // 128 + i] = ch

    @staticmethod
    def _wrap16(idx):
        n = idx.size
        a = idx.astype(np.int16).reshape(n // 16, 16).T
        return np.ascontiguousarray(np.tile(a, (8, 1)))

    def core_inputs(self, k, x, W1, b1, W2, b2, W3, b3):
        sh, shard, t = self.sh, self.shard, self.t
        xs = np.zeros((sh, F), np.float32)
        lo, hi = k * shard, min((k + 1) * shard, self.n_nodes)
        xs[: hi - lo] = x[lo:hi]
        ds = np.zeros(sh, np.float32)
        ds[: hi - lo] = self.dis[lo:hi]
        x_dev = xs.reshape(t, 128, F).transpose(1, 0, 2)
        d_dev = ds.reshape(t, 128).T
        iota = np.broadcast_to(np.arange(128, dtype=np.float16), (128, 128))
        return {
            "x": np.ascontiguousarray(x_dev.reshape(128, t * F)),
            "dis": np.ascontiguousarray(d_dev),
            "gidx": self.gidx[k],
            "dlo": self.dlo[k],
            "iota": np.ascontiguousarray(iota),
            "W1": np.asarray(W1, np.float16),
            "b1": np.asarray(b1, np.float32).reshape(F, 1),
            "W2": np.asarray(W2, np.float16),
            "b2": np.asarray(b2, np.float32).reshape(F, 1),
            "W3": np.asarray(W3, np.float16).reshape(F, 1),
            "b3": np.asarray(b3, np.float32).reshape(1, 1),
        }

    def assemble(self, outs):
        res = np.zeros((self.n_nodes, 1), np.float32)
        for k in range(NCORES):
            o = np.asarray(outs[k]["out"]).reshape(self.sh)
            lo = k * self.shard
            hi = min(lo + self.shard, self.n_nodes)
            res[lo:hi, 0] = o[: hi - lo]
        return res


def build(plan, n_layers=3):
    import concourse.bacc as bacc
    import concourse.mybir as mybir
    import concourse.tile as tile
    from concourse.masks import make_identity

    f32 = mybir.dt.float32
    f8 = mybir.dt.float8e4
    f16 = mybir.dt.float16
    i16 = mybir.dt.int16
    T, SH, NSB = plan.t, plan.sh, plan.nsb
    COLS = plan.cols_max

    nc = bacc.Bacc("TRN2", target_bir_lowering=False, debug=False,
                   num_devices=NCORES, num_swdge_queues=4)

    x_t = nc.dram_tensor("x", [128, T * F], f32, kind="ExternalInput")
    dis_t = nc.dram_tensor("dis", [128, T], f32, kind="ExternalInput")
    gidx_t = nc.dram_tensor("gidx", [128, plan.tot // 16], i16, kind="ExternalInput")
    dlo_t = nc.dram_tensor("dlo", [128, plan.tot // 128], f16, kind="ExternalInput")
    iota_t = nc.dram_tensor("iota", [128, 128], f16, kind="ExternalInput")
    Ws = {}
    for nm, shape, dt in [("W1", [F, F], f16), ("b1", [F, 1], f32),
                          ("W2", [F, F], f16), ("b2", [F, 1], f32),
                          ("W3", [F, 1], f16), ("b3", [1, 1], f32)]:
        Ws[nm] = nc.dram_tensor(nm, shape, dt, kind="ExternalInput")
    out_t = nc.dram_tensor("out", [SH], f32, kind="ExternalOutput")

    g_dram = nc.dram_tensor("g_bounce", [SH * 128], f16, kind="Internal")
    # double-buffered across layers: layer L+1's AllGather must not overwrite
    # the table while layer L's tail gathers still read it (cross-layer WAR
    # on DRAM is not tracked by the scheduler)
    G_bufs = [nc.dram_tensor(f"G_table{i}", [plan.gtbl_rows, 128], f16,
                             kind="Internal", addr_space="Shared")
              for i in range(2)]
    rg = [list(range(NCORES))]
    qt, qrows, chunk_base = plan.qt, plan.qrows, plan.chunk_base

    with tile.TileContext(nc) as tc:
        with tc.tile_pool(name="const", bufs=1) as cpool, \
             tc.tile_pool(name="gst", bufs=1) as gpool, \
             tc.tile_pool(name="msg", bufs=3) as mpool, \
             tc.tile_pool(name="sel", bufs=2) as spool, \
             tc.tile_pool(name="idx", bufs=2) as ipool, \
             tc.tile_pool(name="acc", bufs=2) as apool, \
             tc.tile_pool(name="fm", bufs=2) as fpool, \
             tc.tile_pool(name="pagg", bufs=2, space="PSUM") as pagg, \
             tc.tile_pool(name="ptp", bufs=2, space="PSUM") as ptp, \
             tc.tile_pool(name="pt2", bufs=2, space="PSUM") as pt2p, \
             tc.tile_pool(name="pmm", bufs=2, space="PSUM") as pmm:

            ident = cpool.tile([128, 128], f32)
            make_identity(nc, ident[:])
            ident16 = cpool.tile([128, 128], f16, name="ident16")
            make_identity(nc, ident16[:])
            dis_s = cpool.tile([128, T], f32)
            nc.sync.dma_start(dis_s[:], dis_t[:])
            iota_s = cpool.tile([128, 128], f16, name="iota_s")
            nc.sync.dma_start(iota_s[:], iota_t[:])
            dlo_s = cpool.tile([128, plan.tot // 128], f16, name="dlo_s")
            nc.sync.dma_start(dlo_s[:], dlo_t[:])
            wsb = {}
            for nm in ("W1", "W2", "W3", "b1", "b2", "b3"):
                wsb[nm] = cpool.tile(list(Ws[nm].shape), Ws[nm].dtype,
                                     name=f"sb_{nm}")
                nc.sync.dma_start(wsb[nm][:], Ws[nm][:])

            dis_b = dis_s[:].unsqueeze(-1).broadcast_to([128, T, F])

            xin = mpool.tile([128, T, F], f32, tag="msg")
            nc.sync.dma_start(xin[:], x_t[:].rearrange("p (t f) -> p t f", f=F))
            g = gpool.tile([128, T, 128], f16, tag="g0")
            nc.vector.tensor_tensor(g[:, :, :F], xin[:], dis_b,
                                    mybir.AluOpType.mult)

            g_dram_v = g_dram[:].rearrange("(t p f) -> p t f", p=128, f=128)

            for layer in range(n_layers):
                G = G_bufs[layer % 2]
                # publish g quarters; chunk q of G is its own AllGather
                for q in range(4):
                    t0, t1 = qt[q], qt[q + 1]
                    nc.sync.dma_start(g_dram_v[:, t0:t1, :], g[:, t0:t1, :])
                    nc.gpsimd.collective_compute(
                        "AllGather", mybir.AluOpType.bypass,
                        replica_groups=rg,
                        ins=[g_dram[t0 * 128 * 128:t1 * 128 * 128]],
                        outs=[G[int(chunk_base[q]):
                                int(chunk_base[q]) + NCORES * int(qrows[q]), :]
                              .rearrange("r f -> (r f)")],
                    )

                if layer < n_layers - 1:
                    g_next = gpool.tile([128, T, 128], f16,
                                        tag=f"g{(layer + 1) % 2}")

                acc4 = None
                for sb in range(NSB):
                    base = int(plan.sb_off[sb])
                    tok_sb = int(plan.sb_off[sb + 1]) - base
                    cols_sb = tok_sb // 128
                    gi = ipool.tile([128, plan.tok_sb_max // 16], i16, tag="gi")
                    nc.sync.dma_start(
                        gi[:, : tok_sb // 16],
                        gidx_t[:, base // 16:(base + tok_sb) // 16])
                    msgb = mpool.tile([128, COLS, 128], f16, tag="msg")
                    for ch in range(4):
                        n = int(plan.n_sbc[sb, ch])
                        if n == 0:
                            continue
                        co = (int(plan.off_sbc[sb, ch]) - base) // 128
                        go = (int(plan.off_sbc[sb, ch]) - base) // 16
                        nc.gpsimd.dma_gather(
                            msgb[:, co:co + n // 128, :],
                            G[int(chunk_base[ch]):
                              int(chunk_base[ch]) + NCORES * int(qrows[ch]), :],
                            gi[:, go:go + n // 16],
                            n, n, 128,
                            queue_num=ch,
                            single_packet=bool(n <= 1024))
                    S = spool.tile([128, COLS, 128], f8, tag="S")
                    nc.vector.tensor_tensor(
                        S[:, :cols_sb, :],
                        dlo_s[:, base // 128:base // 128 + cols_sb]
                        .unsqueeze(-1).broadcast_to([128, cols_sb, 128]),
                        iota_s[:].unsqueeze(1).broadcast_to([128, cols_sb, 128]),
                        mybir.AluOpType.is_equal)

                    for t in range(sb * SBS, min((sb + 1) * SBS, T)):
                        cols = plan.groups[t]
                        pa = pagg.tile([128, F], f32, tag="agg")
                        for i, cidx in enumerate(cols):
                            nc.tensor.matmul(
                                pa[:], S[:, cidx, :], msgb[:, cidx, :F],
                                start=(i == 0), stop=(i == len(cols) - 1))
                        j = t % 4
                        if j == 0:
                            acc4 = apool.tile([128, 4, F], f32, tag="acc4")
                        nc.vector.tensor_scalar_mul(
                            acc4[:, j, :], pa[:], dis_s[:, t:t + 1])
                        if j == 3 or t == T - 1:
                            t0 = t - j
                            nt = j + 1
                            fm = fpool.tile([F, 4 * 128], f16, tag="fm")
                            for j2 in range(nt):
                                pt = ptp.tile([F, 128], f32, tag="pt")
                                nc.tensor.transpose(pt[:], acc4[:, j2, :],
                                                    ident[:])
                                nc.vector.tensor_copy(
                                    fm[:, j2 * 128:(j2 + 1) * 128], pt[:])
                            if layer < n_layers - 1:
                                W, b = wsb[f"W{layer + 1}"], wsb[f"b{layer + 1}"]
                                mm = pmm.tile([F, 4 * 128], f32, tag="mm")
                                nc.tensor.matmul(mm[:, : nt * 128], W[:],
                                                 fm[:, : nt * 128],
                                                 start=True, stop=True)
                                hfm = fpool.tile([F, 4 * 128], f16, tag="hfm")
                                nc.scalar.activation(
                                    hfm[:, : nt * 128], mm[:, : nt * 128],
                                    mybir.ActivationFunctionType.Relu,
                                    bias=b[:, :1])
                                for j2 in range(nt):
                                    p2 = pt2p.tile([128, F], f16, tag="pt2")
                                    nc.tensor.transpose(
                                        p2[:], hfm[:, j2 * 128:(j2 + 1) * 128],
                                        ident16[:F, :F])
                                    nc.vector.tensor_scalar_mul(
                                        g_next[:, t0 + j2, :F], p2[:],
                                        dis_s[:, t0 + j2:t0 + j2 + 1])
                            else:
                                W3, b3 = wsb["W3"], wsb["b3"]
                                mm3 = pmm.tile([1, 4 * 128], f32, tag="mm")
                                nc.tensor.matmul(mm3[:, : nt * 128], W3[:],
                                                 fm[:, : nt * 128],
                                                 start=True, stop=True)
                                ofm = fpool.tile([1, 4 * 128], f32, tag="ofm")
                                nc.vector.tensor_scalar_add(
                                    ofm[:, : nt * 128], mm3[:, : nt * 128],
                                    b3[:, :1])
                                nc.sync.dma_start(
                                    out_t[t0 * 128:(t0 + nt) * 128]
                                    .rearrange("(a x) -> a x", a=1),
                                    ofm[:, : nt * 128])
                if layer < n_layers - 1:
                    g = g_next

    nc.compile()
    return nc


def kernel(**inputs):
    import sys
    sys.path.insert(0, "/opt/trn_rl_repo")
    from concourse import bass2jax

    x = np.asarray(inputs["x"], np.float32)
    edge_index = np.asarray(inputs["edge_index"])
    plan = Plan(x.shape[0], edge_index)
    nc = build(plan)
    in_maps = [plan.core_inputs(k, x, inputs["W1"], inputs["b1"], inputs["W2"],
                                inputs["b2"], inputs["W3"], inputs["b3"])
               for k in range(NCORES)]
    results = bass2jax.run_bass_via_pjrt(nc, in_maps, n_cores=NCORES)
    return plan.assemble(results)


# revision 8
# speedup vs baseline: 1.2073x; 1.1190x over previous
"""3-layer GCN on Trainium2, 8 NeuronCores — matmul-aggregation design.

Strategy (graph/data parallel):
  - Nodes block-partitioned across 8 cores (dst-sharded); weights replicated.
  - Per layer each core computes g = dis * h for its shard; shards are
    AllGathered into a shared fp16 HBM table G (rows padded to 256B so
    dma_gather's 256B-element constraint is met).
  - Aggregation: edge tokens are sorted by destination tile (128 dst nodes);
    each token's source row is dma_gathered into a token-major SBUF buffer.
    Per 128-token group, a one-hot selection matrix S (S[tok, dst] =
    (dlo[tok] == dst)) is built on the DVE from a precomputed per-token
    destination id, and the PE accumulates psum[dst, f] += S^T @ msg into
    PSUM. No scatter-add is needed anywhere.
  - Self-loops are extra tokens; padding tokens use dlo=200 so their S row
    is all-zero (gathered garbage contributes nothing).
  - The G table is chunked so gather indices fit int16: chunk q holds
    quarter q of every core's shard, and is filled by its own AllGather so
    gathers of chunk q can start before later chunks arrive.
"""
import sys
import numpy as np

sys.path.insert(0, "/opt/trn_rl_repo")

F = 64
NCORES = 8
SBS = 5            # dst tiles per superblock (gather/matmul pipeline unit)
PAD_DLO = 200.0    # one-hot miss -> padding tokens contribute nothing


class Plan:
    def __init__(self, n_nodes, edge_index):
        self.n_nodes = n_nodes
        shard = (n_nodes + NCORES - 1) // NCORES
        sh = ((shard + 127) // 128) * 128
        self.shard, self.sh, self.t = shard, sh, sh // 128
        T = self.t

        # quarters of each shard (tile-aligned) -> 4 gather chunks
        qt = [0, 25, 50, 75, T]
        self.qt = qt
        qlo = np.array([q * 128 for q in qt[:-1]])
        qrows = np.array([(qt[i + 1] - qt[i]) * 128 for i in range(4)])
        chunk_base = np.concatenate([[0], np.cumsum(qrows * NCORES)])[:4]
        self.qrows, self.chunk_base = qrows, chunk_base
        self.gtbl_rows = int((qrows * NCORES).sum())
        assert self.gtbl_rows == sh * NCORES
        assert all(qrows * NCORES <= 32767)

        row = np.asarray(edge_index[0], dtype=np.int64)
        col = np.asarray(edge_index[1], dtype=np.int64)
        deg = np.bincount(col, minlength=n_nodes).astype(np.float64) + 1.0
        self.dis = (1.0 / np.sqrt(deg)).astype(np.float32)

        def table_pos(src_core, src_l):
            srq = np.minimum(src_l >> 7, T - 1) // 25
            srq = np.minimum(srq, 3)
            inchunk = src_core * qrows[srq] + (src_l - qlo[srq])
            return srq, inchunk

        src_core = row // shard
        src_l = row % shard
        dst_core = col // shard
        cloc = col % shard
        ch_e, ic_e = table_pos(src_core, src_l)

        NSB = (T + SBS - 1) // SBS
        self.nsb = NSB

        # per-core token sets (with self loops)
        per_core = []
        for k in range(NCORES):
            m = dst_core == k
            sc = np.full(shard, k, np.int64)
            sl = np.arange(shard, dtype=np.int64)
            ch_s, ic_s = table_pos(sc, sl)
            ch_k = np.concatenate([ch_e[m], ch_s])
            ic_k = np.concatenate([ic_e[m], ic_s])
            c_k = np.concatenate([cloc[m], sl])
            per_core.append((ch_k, ic_k, c_k))

        # uniform run sizes: n[sb, ch, t] = roundup(max_k count, 128)
        cnt = np.zeros((NCORES, NSB, 4, T), np.int64)
        for k, (ch_k, ic_k, c_k) in enumerate(per_core):
            t_k = c_k >> 7
            sb_k = t_k // SBS
            np.add.at(cnt[k], (sb_k, ch_k, t_k), 1)
        mx = cnt.max(axis=0)
        n_sbct = np.where(mx > 0, ((mx + 127) // 128) * 128, 0)
        self.n_sbct = n_sbct

        # offsets in schedule order (sb, ch, t)
        off_sbct = np.zeros((NSB, 4, T), np.int64)
        self.sb_off = np.zeros(NSB + 1, np.int64)
        self.n_sbc = np.zeros((NSB, 4), np.int64)
        self.off_sbc = np.zeros((NSB, 4), np.int64)
        o = 0
        for sb in range(NSB):
            self.sb_off[sb] = o
            t0, t1 = sb * SBS, min((sb + 1) * SBS, T)
            for ch in range(4):
                self.off_sbc[sb, ch] = o
                for t in range(t0, t1):
                    off_sbct[sb, ch, t] = o
                    o += int(n_sbct[sb, ch, t])
                self.n_sbc[sb, ch] = o - self.off_sbc[sb, ch]
        self.sb_off[NSB] = o
        self.tot = o
        self.tok_sb_max = int((self.sb_off[1:] - self.sb_off[:-1]).max())
        self.cols_max = self.tok_sb_max // 128

        # per-tile matmul groups: column indices relative to the sb base
        self.groups = []          # [t] -> list of sb-relative col indices
        for t in range(T):
            sb = t // SBS
            base = self.sb_off[sb]
            cols = []
            for ch in range(4):
                go = (off_sbct[sb, ch, t] - base) // 128
                for i in range(int(n_sbct[sb, ch, t]) // 128):
                    cols.append(int(go + i))
            self.groups.append(cols)

        # per-core gather index + dlo tables
        self.gidx = []
        self.dlo = []
        for k, (ch_k, ic_k, c_k) in enumerate(per_core):
            t_k = c_k >> 7
            sb_k = t_k // SBS
            key = (sb_k * 4 + ch_k) * T + t_k
            order = np.argsort(key, kind="stable")
            ks = key[order]
            gflat = np.zeros(self.tot, np.int64)
            dflat = np.full(self.tot, PAD_DLO, np.float32)
            if ks.size:
                starts = np.r_[True, ks[1:] != ks[:-1]]
                run_starts = np.flatnonzero(starts)
                rid = np.cumsum(starts) - 1
                within = np.arange(ks.size) - run_starts[rid]
                sbv = ks // (4 * T)
                chv = (ks // T) % 4
                tv = ks % T
                pos = off_sbct[sbv, chv, tv] + within
                e = order
                gflat[pos] = ic_k[e]
                dflat[pos] = (c_k[e] & 127).astype(np.float32)
            self.gidx.append(self._wrap16(gflat))
            self.dlo.append(np.ascontiguousarray(
                dflat.reshape(self.tot // 128, 128).T.astype(np.float16)))
            self._gflat_dbg = getattr(self, "_gflat_dbg", [])
            self._gflat_dbg.append(gflat)
            self._dflat_dbg = getattr(self, "_dflat_dbg", [])
            self._dflat_dbg.append(dflat)

        # debug: map absolute token-col -> (tile, chunk); -1 where unused
        self.colmap = np.full(self.tot // 128, -1, np.int64)
        self.colch = np.full(self.tot // 128, -1, np.int64)
        for t in range(T):
            sb = t // SBS
            for ch in range(4):
                go = off_sbct[sb, ch, t]
                for i in range(int(n_sbct[sb, ch, t]) // 128):
                    self.colmap[go // 128 + i] = t
                    self.colch[go // 128 + i] = ch

    @staticmethod
    def _wrap16(idx):
        n = idx.size
        a = idx.astype(np.int16).reshape(n // 16, 16).T
        return np.ascontiguousarray(np.tile(a, (8, 1)))

    def core_inputs(self, k, x, W1, b1, W2, b2, W3, b3):
        sh, shard, t = self.sh, self.shard, self.t
        xs = np.zeros((sh, F), np.float32)
        lo, hi = k * shard, min((k + 1) * shard, self.n_nodes)
        xs[: hi - lo] = x[lo:hi]
        ds = np.zeros(sh, np.float32)
        ds[: hi - lo] = self.dis[lo:hi]
        x_dev = xs.reshape(t, 128, F).transpose(1, 0, 2)
        d_dev = ds.reshape(t, 128).T
        iota = np.broadcast_to(np.arange(128, dtype=np.float16), (128, 128))
        return {
            "x": np.ascontiguousarray(x_dev.reshape(128, t * F)),
            "dis": np.ascontiguousarray(d_dev),
            "gidx": self.gidx[k],
            "dlo": self.dlo[k],
            "iota": np.ascontiguousarray(iota),
            "W1": np.asarray(W1, np.float16),
            "b1": np.asarray(b1, np.float32).reshape(F, 1),
            "W2": np.asarray(W2, np.float16),
            "b2": np.asarray(b2, np.float32).reshape(F, 1),
            "W3": np.asarray(W3, np.float16).reshape(F, 1),
            "b3": np.asarray(b3, np.float32).reshape(1, 1),
        }

    def assemble(self, outs):
        res = np.zeros((self.n_nodes, 1), np.float32)
        for k in range(NCORES):
            o = np.asarray(outs[k]["out"]).reshape(self.sh)
            lo = k * self.shard
            hi = min(lo + self.shard, self.n_nodes)
            res[lo:hi, 0] = o[: hi - lo]
        return res


def build(plan, n_layers=3):
    import concourse.bacc as bacc
    import concourse.mybir as mybir
    import concourse.tile as tile
    from concourse.masks import make_identity

    f32 = mybir.dt.float32
    f8 = mybir.dt.float8e4
    f16 = mybir.dt.float16
    i16 = mybir.dt.int16
    T, SH, NSB = plan.t, plan.sh, plan.nsb
    COLS = plan.cols_max

    nc = bacc.Bacc("TRN2", target_bir_lowering=False, debug=False,
                   num_devices=NCORES, num_swdge_queues=4)

    x_t = nc.dram_tensor("x", [128, T * F], f32, kind="ExternalInput")
    dis_t = nc.dram_tensor("dis", [128, T], f32, kind="ExternalInput")
    gidx_t = nc.dram_tensor("gidx", [128, plan.tot // 16], i16, kind="ExternalInput")
    dlo_t = nc.dram_tensor("dlo", [128, plan.tot // 128], f16, kind="ExternalInput")
    iota_t = nc.dram_tensor("iota", [128, 128], f16, kind="ExternalInput")
    Ws = {}
    for nm, shape, dt in [("W1", [F, F], f16), ("b1", [F, 1], f32),
                          ("W2", [F, F], f16), ("b2", [F, 1], f32),
                          ("W3", [F, 1], f16), ("b3", [1, 1], f32)]:
        Ws[nm] = nc.dram_tensor(nm, shape, dt, kind="ExternalInput")
    out_t = nc.dram_tensor("out", [SH], f32, kind="ExternalOutput")

    g_dram = nc.dram_tensor("g_bounce", [SH * 128], f16, kind="Internal")
    # S depends only on the graph: build on DVE in layer 0, spill to HBM,
    # DMA-reload in later layers (sync-engine FIFO orders store before load)
    S_dram = nc.dram_tensor("S_spill", [plan.tot * 128], f8, kind="Internal")
    # double-buffered across layers: layer L+1's AllGather must not overwrite
    # the table while layer L's tail gathers still read it (cross-layer WAR
    # on DRAM is not tracked by the scheduler)
    G_bufs = [nc.dram_tensor(f"G_table{i}", [plan.gtbl_rows, 128], f16,
                             kind="Internal", addr_space="Shared")
              for i in range(2)]
    rg = [list(range(NCORES))]
    qt, qrows, chunk_base = plan.qt, plan.qrows, plan.chunk_base

    with tile.TileContext(nc) as tc:
        with tc.tile_pool(name="const", bufs=1) as cpool, \
             tc.tile_pool(name="gst", bufs=1) as gpool, \
             tc.tile_pool(name="msg", bufs=3) as mpool, \
             tc.tile_pool(name="sel", bufs=2) as spool, \
             tc.tile_pool(name="idx", bufs=2) as ipool, \
             tc.tile_pool(name="acc", bufs=2) as apool, \
             tc.tile_pool(name="fm", bufs=2) as fpool, \
             tc.tile_pool(name="pagg", bufs=2, space="PSUM") as pagg, \
             tc.tile_pool(name="ptp", bufs=2, space="PSUM") as ptp, \
             tc.tile_pool(name="pt2", bufs=2, space="PSUM") as pt2p, \
             tc.tile_pool(name="pmm", bufs=2, space="PSUM") as pmm:

            ident = cpool.tile([128, 128], f32)
            make_identity(nc, ident[:])
            ident16 = cpool.tile([128, 128], f16, name="ident16")
            make_identity(nc, ident16[:])
            dis_s = cpool.tile([128, T], f32)
            nc.sync.dma_start(dis_s[:], dis_t[:])
            iota_s = cpool.tile([128, 128], f16, name="iota_s")
            nc.sync.dma_start(iota_s[:], iota_t[:])
            dlo_s = cpool.tile([128, plan.tot // 128], f16, name="dlo_s")
            nc.sync.dma_start(dlo_s[:], dlo_t[:])
            wsb = {}
            for nm in ("W1", "W2", "W3", "b1", "b2", "b3"):
                wsb[nm] = cpool.tile(list(Ws[nm].shape), Ws[nm].dtype,
                                     name=f"sb_{nm}")
                nc.sync.dma_start(wsb[nm][:], Ws[nm][:])

            dis_b = dis_s[:].unsqueeze(-1).broadcast_to([128, T, F])

            xin = mpool.tile([128, T, F], f32, tag="msg")
            nc.sync.dma_start(xin[:], x_t[:].rearrange("p (t f) -> p t f", f=F))
            g = gpool.tile([128, T, 128], f16, tag="g0")
            nc.vector.tensor_tensor(g[:, :, :F], xin[:], dis_b,
                                    mybir.AluOpType.mult)

            g_dram_v = g_dram[:].rearrange("(t p f) -> p t f", p=128, f=128)

            for layer in range(n_layers):
                G = G_bufs[layer % 2]
                # publish g quarters; chunk q of G is its own AllGather
                for q in range(4):
                    t0, t1 = qt[q], qt[q + 1]
                    nc.sync.dma_start(g_dram_v[:, t0:t1, :], g[:, t0:t1, :])
                    nc.gpsimd.collective_compute(
                        "AllGather", mybir.AluOpType.bypass,
                        replica_groups=rg,
                        ins=[g_dram[t0 * 128 * 128:t1 * 128 * 128]],
                        outs=[G[int(chunk_base[q]):
                                int(chunk_base[q]) + NCORES * int(qrows[q]), :]
                              .rearrange("r f -> (r f)")],
                    )

                if layer < n_layers - 1:
                    g_next = gpool.tile([128, T, 128], f16,
                                        tag=f"g{(layer + 1) % 2}")

                acc4 = None
                for sb in range(NSB):
                    base = int(plan.sb_off[sb])
                    tok_sb = int(plan.sb_off[sb + 1]) - base
                    cols_sb = tok_sb // 128
                    gi = ipool.tile([128, plan.tok_sb_max // 16], i16, tag="gi")
                    nc.sync.dma_start(
                        gi[:, : tok_sb // 16],
                        gidx_t[:, base // 16:(base + tok_sb) // 16])
                    msgb = mpool.tile([128, COLS, 128], f16, tag="msg")
                    for ch in range(4):
                        n = int(plan.n_sbc[sb, ch])
                        if n == 0:
                            continue
                        co = (int(plan.off_sbc[sb, ch]) - base) // 128
                        go = (int(plan.off_sbc[sb, ch]) - base) // 16
                        nc.gpsimd.dma_gather(
                            msgb[:, co:co + n // 128, :],
                            G[int(chunk_base[ch]):
                              int(chunk_base[ch]) + NCORES * int(qrows[ch]), :],
                            gi[:, go:go + n // 16],
                            n, n, 128,
                            queue_num=ch,
                            single_packet=bool(n <= 1024))
                    S = spool.tile([128, COLS, 128], f8, tag="S")
                    S_dram_v = S_dram[:].rearrange(
                        "(c p d) -> p c d", p=128, d=128)
                    scol = base // 128
                    if layer == 0:
                        nc.vector.tensor_tensor(
                            S[:, :cols_sb, :],
                            dlo_s[:, scol:scol + cols_sb]
                            .unsqueeze(-1).broadcast_to([128, cols_sb, 128]),
                            iota_s[:].unsqueeze(1)
                            .broadcast_to([128, cols_sb, 128]),
                            mybir.AluOpType.is_equal)
                        nc.sync.dma_start(
                            S_dram_v[:, scol:scol + cols_sb, :],
                            S[:, :cols_sb, :])
                    else:
                        # scalar (ACT) engine = second HWDGE issue path;
                        # keeps S reloads off the busy SP DMA queue
                        nc.scalar.dma_start(
                            S[:, :cols_sb, :],
                            S_dram_v[:, scol:scol + cols_sb, :])

                    for t in range(sb * SBS, min((sb + 1) * SBS, T)):
                        cols = plan.groups[t]
                        pa = pagg.tile([128, F], f32, tag="agg")
                        for i, cidx in enumerate(cols):
                            nc.tensor.matmul(
                                pa[:], S[:, cidx, :], msgb[:, cidx, :F],
                                start=(i == 0), stop=(i == len(cols) - 1))
                        j = t % 4
                        if j == 0:
                            acc4 = apool.tile([128, 4, F], f32, tag="acc4")
                        nc.vector.tensor_scalar_mul(
                            acc4[:, j, :], pa[:], dis_s[:, t:t + 1])
                        if j == 3 or t == T - 1:
                            t0 = t - j
                            nt = j + 1
                            fm = fpool.tile([F, 4 * 128], f16, tag="fm")
                            for j2 in range(nt):
                                pt = ptp.tile([F, 128], f32, tag="pt")
                                nc.tensor.transpose(pt[:], acc4[:, j2, :],
                                                    ident[:])
                                nc.vector.tensor_copy(
                                    fm[:, j2 * 128:(j2 + 1) * 128], pt[:])
                            if layer < n_layers - 1:
                                W, b = wsb[f"W{layer + 1}"], wsb[f"b{layer + 1}"]
                                mm = pmm.tile([F, 4 * 128], f32, tag="mm")
                                nc.tensor.matmul(mm[:, : nt * 128], W[:],
                                                 fm[:, : nt * 128],
                                                 start=True, stop=True)
                                hfm = fpool.tile([F, 4 * 128], f16, tag="hfm")
                                nc.scalar.activation(
                                    hfm[:, : nt * 128], mm[:, : nt * 128],
                                    mybir.ActivationFunctionType.Relu,
                                    bias=b[:, :1])
                                for j2 in range(nt):
                                    p2 = pt2p.tile([128, F], f16, tag="pt2")
                                    nc.tensor.transpose(
                                        p2[:], hfm[:, j2 * 128:(j2 + 1) * 128],
                                        ident16[:F, :F])
                                    nc.vector.tensor_scalar_mul(
                                        g_next[:, t0 + j2, :F], p2[:],
                                        dis_s[:, t0 + j2:t0 + j2 + 1])
                            else:
                                W3, b3 = wsb["W3"], wsb["b3"]
                                mm3 = pmm.tile([1, 4 * 128], f32, tag="mm")
                                nc.tensor.matmul(mm3[:, : nt * 128], W3[:],
                                                 fm[:, : nt * 128],
                                                 start=True, stop=True)
                                ofm = fpool.tile([1, 4 * 128], f32, tag="ofm")
                                nc.vector.tensor_scalar_add(
                                    ofm[:, : nt * 128], mm3[:, : nt * 128],
                                    b3[:, :1])
                                nc.sync.dma_start(
                                    out_t[t0 * 128:(t0 + nt) * 128]
                                    .rearrange("(a x) -> a x", a=1),
                                    ofm[:, : nt * 128])
                if layer < n_layers - 1:
                    g = g_next

    nc.compile()
    return nc


def kernel(**inputs):
    import sys
    sys.path.insert(0, "/opt/trn_rl_repo")
    from concourse import bass2jax

    x = np.asarray(inputs["x"], np.float32)
    edge_index = np.asarray(inputs["edge_index"])
    plan = Plan(x.shape[0], edge_index)
    nc = build(plan)
    in_maps = [plan.core_inputs(k, x, inputs["W1"], inputs["b1"], inputs["W2"],
                                inputs["b2"], inputs["W3"], inputs["b3"])
               for k in range(NCORES)]
    results = bass2jax.run_bass_via_pjrt(nc, in_maps, n_cores=NCORES)
    return plan.assemble(results)
